# revision 1
# baseline (speedup 1.0000x reference)
"""Self-contained Trainium2 Bass kernel for nn_Discriminator_51049981280755.

Strategy: data-parallel over batch (8 cores, 1 batch element each, no
collectives). Per core: full transformer forward in fp32r with bf16 on the
embedding, attention-output and FFN paths. Residual x kept feature-major
[D, T] resident in SBUF; linear attention via per-head-pair block-diagonal
KV matrices; final layer computes only token 0 (the CLS readout).

v2 vs v1: host-transposed bf16 input + SBUF-resident bf16 We for the
embedding (no PE transposes, ~5x less DMA); balanced 6-chunk token grid
(no degenerate 2-wide tail chunk); KV projections with k|v fused into one
moving operand so each x-stationary load serves both; rotate-half via a PE
permutation matmul instead of latency-heavy gpsimd DMAs; stationary-weight
reuse (d-/f-outer accumulation) in FFN to hide LDWEIGHTS.
"""
import numpy as np
import ml_dtypes
import concourse.bass as bass
import concourse.tile as tile
from concourse import mybir
import concourse.bass_utils as bass_utils

bass_utils.upload_artifacts = lambda d: str(d)  # no S3 in this container

F32 = mybir.dt.float32
F32R = mybir.dt.float32r
BF16 = mybir.dt.bfloat16
AF = mybir.ActivationFunctionType
ALU = mybir.AluOpType
P = 128


def legalize_wait_counts(nc, max_waits=1):
    """This walrus build rejects instructions carrying more than one sync-wait
    command. Hoist excess waits onto preceding same-engine NoOps."""
    n_split = 0
    for fn in nc.m.functions:
        for blk in fn.blocks:
            insts = blk.instructions
            need = False
            for inst in insts:
                si = inst.sync_info
                if si is not None and len(si.on_wait) > max_waits:
                    need = True
                    break
            if not need:
                continue
            out = []
            for inst in insts:
                si = inst.sync_info
                if si is not None and len(si.on_wait) > max_waits:
                    waits = list(si.on_wait)
                    extra, keep = waits[:-max_waits], waits[-max_waits:]
                    for j in range(0, len(extra), max_waits):
                        nop = mybir.InstNoOp(
                            name=nc.get_next_instruction_name(),
                            ins=[], outs=[], text_hint="waitsplit",
                        )
                        nop.engine = inst.engine
                        nop.sync_info = mybir.SyncInfo(
                            on_wait=list(extra[j:j + max_waits]), on_update=[])
                        out.append(nop)
                        n_split += 1
                    inst.sync_info = mybir.SyncInfo(
                        on_wait=keep, on_update=list(si.on_update))
                out.append(inst)
            blk.instructions = out
    return n_split


class Cfg:
    def __init__(self, SEQ=2048, V=4096, D=1024, FF=4096, NL=8):
        self.SEQ, self.V, self.D, self.FF, self.NL = SEQ, V, D, FF, NL
        self.T = SEQ + 1
        self.KD = D // P            # d-tiles (= head pairs)
        self.KF = FF // P
        self.KV = V // P
        # scr padding: bf16 view of scr must hold 4 h1 slices of >= half width
        self.TP = 2052
        self.h1w = (2 * self.TP) // 4          # 1026
        # balanced token chunks, all even widths (fp32r matmul ISA needs even
        # free dims); last real token 2048 plus one zero-padded column 2049
        self.chunks = [(0, 342), (342, 342), (684, 342),
                       (1026, 342), (1368, 340), (1708, 342)]
        assert sum(w for _, w in self.chunks) == self.T + 1
        self.half_chunks = [self.chunks[:3], self.chunks[3:]]
        self.halves = [(0, 1026), (1026, 1024)]
        # s-tiles for KV phase (token-major row tiles)
        nst_full = self.T // P
        self.tail_rows = self.T - nst_full * P
        self.NST = nst_full + (1 if self.tail_rows else 0)

    def vecs_layout(self):
        idx = {}
        n = 0
        for l in range(self.NL):
            for nm in ("ln1g", "ln2g"):
                for d in range(self.KD):
                    idx[(nm, l, d)] = n; n += 1
        for d in range(self.KD):
            idx[("gN", 0, d)] = n; n += 1
        for d in range(self.KD):
            idx[("cls", 0, d)] = n; n += 1
        idx[("epsln", 0, 0)] = n; n += 1
        idx[("epsat", 0, 0)] = n; n += 1
        self.NVEC = n
        return idx


def host_prep(C, weights):
    """Build host-side constant arrays. weights: dict from setup_inputs()."""
    T, TP, D = C.T, C.TP, C.D
    f32 = np.float32
    out = {}
    # rotary tables
    inv_freq = 1.0 / (10000.0 ** (np.arange(0, 64, 2, dtype=np.float64) / 64))
    freqs = np.arange(T, dtype=np.float64)[:, None] * inv_freq[None, :]   # [T, 32]
    emb = np.concatenate([freqs, freqs], axis=-1)                          # [T, 64]
    cos = np.cos(emb).astype(f32)
    sin = np.sin(emb).astype(f32)
    sinS = sin.copy()
    sinS[:, :32] *= -1.0
    # feature-major [128, TP]: row p -> dh = p % 64, col = s
    cos_fm = np.zeros((P, TP), f32)
    sinS_fm = np.zeros((P, TP), f32)
    for p in range(P):
        cos_fm[p, :T] = cos[:, p % 64]
        sinS_fm[p, :T] = sinS[:, p % 64]
    # token-major [128, NST, 64]: row p, stile j -> s = 128j + p
    cos_tm = np.zeros((P, C.NST, 64), f32)
    sinS_tm = np.zeros((P, C.NST, 64), f32)
    for j in range(C.NST):
        s0 = j * P
        rows = min(P, T - s0)
        cos_tm[:rows, j, :] = cos[s0:s0 + rows]
        sinS_tm[:rows, j, :] = sinS[s0:s0 + rows]
    out["cos_fm"], out["sinS_fm"] = cos_fm, sinS_fm
    out["cos_tm"] = cos_tm.reshape(P, C.NST * 64).copy()
    out["sinS_tm"] = sinS_tm.reshape(P, C.NST * 64).copy()
    # rotate-half permutation: out = Rsw.T @ q  ->  out[q_] = q[swap(q_)]
    rsw = np.zeros((P, P), f32)
    for q_ in range(P):
        blk = (q_ // 64) * 64
        r = q_ % 64
        src = blk + (r + 32 if r < 32 else r - 32)
        rsw[src, q_] = 1.0
    out["rsw"] = rsw
    # posbe [D, TP] = pos[:T].T + be
    posbe = np.zeros((D, TP), f32)
    posbe[:, :T] = np.asarray(weights["pos"][:T], f32).T + np.asarray(weights["be"], f32)[:, None]
    out["posbe"] = posbe
    # vecs
    idx = C.vecs_layout()
    vecs = np.zeros((P, C.NVEC), f32)
    ln1g = np.asarray(weights["ln1g"], f32)
    ln2g = np.asarray(weights["ln2g"], f32)
    for l in range(C.NL):
        for d in range(C.KD):
            vecs[:, idx[("ln1g", l, d)]] = ln1g[l, d * P:(d + 1) * P]
            vecs[:, idx[("ln2g", l, d)]] = ln2g[l, d * P:(d + 1) * P]
    gN = np.asarray(weights["gN"], f32)
    cls = (np.asarray(weights["We"], f32)[C.V - 1] + np.asarray(weights["be"], f32)
           + np.asarray(weights["pos"], f32)[0])
    for d in range(C.KD):
        vecs[:, idx[("gN", 0, d)]] = gN[d * P:(d + 1) * P]
        vecs[:, idx[("cls", 0, d)]] = cls[d * P:(d + 1) * P]
    vecs[:, idx[("epsln", 0, 0)]] = 1e-5
    vecs[:, idx[("epsat", 0, 0)]] = 1e-6
    out["vecs"] = vecs
    e2 = np.zeros((2, P), f32)
    e2[0, 0:64] = 1.0
    e2[1, 64:128] = 1.0
    out["e2"] = e2
    # weights
    out["We"] = np.ascontiguousarray(np.asarray(weights["We"], f32)
                                     .astype(ml_dtypes.bfloat16))
    for nm in ("Wq", "Wk", "Wv", "W1"):
        out[nm.lower()] = np.ascontiguousarray(np.asarray(weights[nm], f32))
    out["wo"] = np.ascontiguousarray(
        np.asarray(weights["Wo"], f32).astype(ml_dtypes.bfloat16))
    out["w2"] = np.ascontiguousarray(
        np.asarray(weights["W2"], f32).astype(ml_dtypes.bfloat16))
    out["wout"] = np.ascontiguousarray(np.asarray(weights["Wout"], f32))
    out["bout"] = np.ascontiguousarray(np.asarray(weights["bout"], f32)[None, :])
    return out


def build(nc, C, dbg=()):
    """Emit the full forward kernel. Returns dict of DRAM tensor handles."""
    from contextlib import ExitStack
    T, TP, D, KD, KF, KVt, NL = C.T, C.TP, C.D, C.KD, C.KF, C.KV, C.NL
    idx = C.vecs_layout()
    dd = {}

    def din(name, shape, dt=F32):
        dd[name] = nc.dram_tensor(name, shape, dt, kind="ExternalInput")
        return dd[name]

    xinT_d = din("xinT", [C.V, C.SEQ], BF16)
    We_d = din("We", [C.V, D], BF16)
    posbe_d = din("posbe", [D, TP])
    cosfm_d = din("cos_fm", [P, TP]); sinSfm_d = din("sinS_fm", [P, TP])
    costm_d = din("cos_tm", [P, C.NST * 64]); sinStm_d = din("sinS_tm", [P, C.NST * 64])
    vecs_d = din("vecs", [P, C.NVEC])
    rsw_d = din("rsw", [P, P])
    e2_d = din("e2", [2, P])
    wq_d = din("wq", [NL, D, D]); wk_d = din("wk", [NL, D, D])
    wv_d = din("wv", [NL, D, D]); wo_d = din("wo", [NL, D, D], BF16)
    w1_d = din("w1", [NL, D, C.FF])
    w2_d = din("w2", [NL, C.FF, D], BF16)
    wout_d = din("wout", [D, C.V])
    bout_d = din("bout", [1, C.V])
    y_d = nc.dram_tensor("y", [1, C.V], F32, kind="ExternalOutput")
    dd["y"] = y_d
    dbg_d = {}
    for nm in dbg:
        dbg_d[nm] = nc.dram_tensor("dbg_" + nm, [C.KD * P, TP], F32,
                                   kind="ExternalOutput")
        dd["dbg_" + nm] = dbg_d[nm]

    with ExitStack() as ctx:
        tc = ctx.enter_context(tile.TileContext(nc))
        big = ctx.enter_context(tc.tile_pool(name="big", bufs=1))
        wkvp = ctx.enter_context(tc.tile_pool(name="wkvp", bufs=2))
        wp = ctx.enter_context(tc.tile_pool(name="wp", bufs=4))
        tmp = ctx.enter_context(tc.tile_pool(name="tmp", bufs=10))
        pp = ctx.enter_context(tc.tile_pool(name="pp", bufs=8, space="PSUM"))

        WKV = lambda: wkvp.tile([P, KD, 2 * P], F32R, tag="wkv", name="wkv")
        W = lambda shape, dt=F32R: wp.tile(shape, dt, tag="w", name="w")
        TT = lambda shape=None, dt=F32: tmp.tile(shape or [P, 344], dt, tag="t", name="t")
        PS = lambda shape=None: pp.tile(shape or [P, 512], F32, tag="p", name="p")

        x = [big.tile([P, TP], F32R, tag=f"x{d}", name=f"x{d}") for d in range(KD)]
        scr = [big.tile([P, TP], F32R, tag=f"scr{d}", name=f"scr{d}") for d in range(KD)]
        cos_fm = big.tile([P, TP], F32, tag="cosfm")
        sinS_fm = big.tile([P, TP], F32, tag="sinSfm")
        cos_tm = big.tile([P, C.NST, 64], F32, tag="costm")
        sinS_tm = big.tile([P, C.NST, 64], F32, tag="sinStm")
        vecs = big.tile([P, C.NVEC], F32, tag="vecs")
        rsw = big.tile([P, P], F32R, tag="rsw")
        e2 = big.tile([2, P], F32R, tag="e2")
        zero128 = big.tile([P, P], F32R, tag="zero128")
        ones_col = big.tile([P, 1], F32R, tag="ones_col")
        ones_col2 = big.tile([P, 2], F32R, tag="ones_col2")
        ones_row = big.tile([1, P], F32R, tag="ones_row")
        kvbd = [big.tile([P, P], F32R, tag=f"kvbd{p_}", name=f"kvbd{p_}") for p_ in range(KD)]
        ksel = [big.tile([P, 2], F32R, tag=f"ksel{p_}", name=f"ksel{p_}") for p_ in range(KD)]

        nc.sync.dma_start(cos_fm[:], cosfm_d[:])
        nc.sync.dma_start(sinS_fm[:], sinSfm_d[:])
        nc.sync.dma_start(cos_tm[:], costm_d.rearrange("p (j e) -> p j e", e=64))
        nc.sync.dma_start(sinS_tm[:], sinStm_d.rearrange("p (j e) -> p j e", e=64))
        nc.sync.dma_start(vecs[:], vecs_d[:])
        nc.sync.dma_start(rsw[:], rsw_d[:].bitcast(F32R))
        nc.sync.dma_start(e2[:], e2_d[:].bitcast(F32R))
        nc.vector.memset(zero128.bitcast(F32)[:], 0.0)
        nc.vector.memset(ones_col.bitcast(F32)[:], 1.0)
        nc.vector.memset(ones_col2.bitcast(F32)[:], 1.0)
        nc.vector.memset(ones_row.bitcast(F32)[:], 1.0)
        for d in range(KD):
            nc.vector.tensor_copy(x[d][:, T:TP],
                                  zero128.bitcast(F32)[:, 0:TP - T])

        vcol = lambda nm, l=0, d=0: vecs[:, idx[(nm, l, d)]:idx[(nm, l, d)] + 1]

        def dump(nm):
            if nm not in dbg_d:
                return
            t = dbg_d[nm]
            for d in range(KD):
                nc.sync.dma_start(t[d * P:(d + 1) * P, :], x[d].bitcast(F32)[:])

        xf = [t.bitcast(F32) for t in x]
        scrb = [t.bitcast(BF16) for t in scr]

        def ln_pass(chunks_list, gname, l):
            """in-place layernorm on x over the given column chunks,
            software-pipelined one chunk deep."""
            invD = 1.0 / D
            nch = len(chunks_list)
            pend = []

            def emit_sums(c0, cw):
                cs = slice(c0, c0 + cw)
                sums = PS(); sq = PS()
                for d in range(KD):
                    nc.tensor.matmul(sums[0:1, :cw], ones_col[:], x[d][:, cs],
                                     start=(d == 0), stop=(d == KD - 1))
                for d in range(KD):
                    x2t = TT(dt=F32R)
                    nc.scalar.activation(x2t[:, 0:cw], xf[d][:, cs], AF.Square)
                    nc.tensor.matmul(sq[0:1, :cw], ones_col[:], x2t[:, 0:cw],
                                     start=(d == 0), stop=(d == KD - 1))
                return sums, sq

            def emit_norm(c0, cw, sums, sq):
                cs = slice(c0, c0 + cw)
                meant = TT(); m2tt = TT(); vart = TT()
                mean = meant.bitcast(F32)[0:1, 0:cw]
                m2t = m2tt.bitcast(F32)[0:1, 0:cw]
                var = vart.bitcast(F32)[0:1, 0:cw]
                nc.vector.tensor_scalar_mul(mean, sums[0:1, :cw], invD)
                nc.scalar.activation(m2t, mean, AF.Square)
                nc.vector.scalar_tensor_tensor(var, sq[0:1, :cw], invD, m2t,
                                               ALU.mult, ALU.subtract)
                rstdt = TT(dt=F32R); m2rt = TT(dt=F32R); lnvt = TT()
                rstdf = rstdt.bitcast(F32)[0:1, 0:cw]
                rstd = rstdt[0:1, 0:cw]
                m2 = m2rt[0:1, 0:cw]
                lnv = lnvt.bitcast(F32)[0:1, 0:cw]
                nc.scalar.activation(lnv, var, AF.Ln, bias=vcol("epsln")[0:1, :])
                nc.scalar.activation(rstd, lnv, AF.Exp, scale=-0.5)
                nc.vector.scalar_tensor_tensor(m2, mean, -1.0, rstdf, ALU.mult, ALU.mult)
                Rrep = PS(); Mrep = PS()
                nc.tensor.matmul(Rrep[:, :cw], ones_row[:], rstd, start=True, stop=True)
                nc.tensor.matmul(Mrep[:, :cw], ones_row[:], m2, start=True, stop=True)
                for d in range(KD):
                    g = vcol(gname, l, d)
                    t1 = TT()
                    nc.vector.scalar_tensor_tensor(t1.bitcast(F32)[:, 0:cw], xf[d][:, cs], g,
                                                   Rrep[:, :cw], ALU.mult, ALU.mult)
                    nc.vector.scalar_tensor_tensor(x[d][:, cs], Mrep[:, :cw], g,
                                                   t1.bitcast(F32)[:, 0:cw], ALU.mult, ALU.add)

            for i, (c0, cw) in enumerate(chunks_list):
                pend.append((c0, cw) + emit_sums(c0, cw))
                if len(pend) > 1:
                    emit_norm(*pend.pop(0))
            while pend:
                emit_norm(*pend.pop(0))

        # ---------------- embedding ----------------
        with nc.named_scope("emb"):
            # We resident as bf16 in the (currently unused) scr region:
            # We tile vt -> scr[vt//4] bf16 cols [(vt%4)*1024, +1024)
            for vt in range(KVt):
                nc.sync.dma_start(
                    scrb[vt // 4][:, (vt % 4) * 1024:(vt % 4 + 1) * 1024],
                    We_d[vt * P:(vt + 1) * P, :])
            for d in range(KD):
                nc.scalar.copy(x[d][:, 0:1], vcol("cls", 0, d))
            for sc in range(C.SEQ // 512):
                embps = [PS() for _ in range(KD)]
                for vt in range(KVt):
                    xt = wp.tile([P, 512], BF16, tag="w", name="xt")
                    nc.sync.dma_start(xt[:], xinT_d[vt * P:(vt + 1) * P,
                                      sc * 512:(sc + 1) * 512])
                    for d in range(KD):
                        wsl = scrb[vt // 4][:, (vt % 4) * 1024 + d * P:
                                           (vt % 4) * 1024 + (d + 1) * P]
                        nc.tensor.matmul(embps[d][:, :], wsl, xt[:],
                                         start=(vt == 0), stop=(vt == KVt - 1))
                cs = slice(1 + sc * 512, 1 + (sc + 1) * 512)
                for d in range(KD):
                    pb = wp.tile([P, 512], F32, tag="w", name="pb")
                    nc.sync.dma_start(pb[:], posbe_d[d * P:(d + 1) * P, cs])
                    nc.vector.tensor_tensor(x[d][:, cs], embps[d][:, :],
                                            pb[:], ALU.add)

        dump("emb")
        # ---------------- layers ----------------
        for l in range(NL):
            last = (l == NL - 1)
            lchunks = [(0, 2)] if last else C.chunks

            # ---- KV phase: k/v token-major -> kvbd (blockdiag) + ksel ----
            # 4 rounds of 2 head-pairs: wider rotary ops, half the DVE iters
            with nc.named_scope(f"l{l}_kv"):
                for rr in range(KD // 2):
                    wkk = WKV()
                    wvv = WKV()
                    for half in range(2):
                        mtq = 2 * rr + half
                        nc.sync.dma_start(
                            wkk[:, :, half * P:(half + 1) * P],
                            wk_d[l, :, mtq * P:(mtq + 1) * P]
                            .rearrange("(a p) f -> p a f", p=P).bitcast(F32R))
                        nc.sync.dma_start(
                            wvv[:, :, half * P:(half + 1) * P],
                            wv_d[l, :, mtq * P:(mtq + 1) * P]
                            .rearrange("(a p) f -> p a f", p=P).bitcast(F32R))
                    kvx = [PS([P, P]) for _ in range(2)]
                    ksp = [PS([P, 2]) for _ in range(2)]
                    for st in range(C.NST):
                        rows = P if st * P + P <= T else T - st * P
                        rs = slice(st * P, st * P + rows)
                        kp = PS([P, 2 * P])
                        vp = PS([P, 2 * P])
                        for d in range(KD):
                            nc.tensor.matmul(kp[:rows, :], x[d][:, rs],
                                             wkk[:, d, :],
                                             start=(d == 0), stop=(d == KD - 1))
                            nc.tensor.matmul(vp[:rows, :], x[d][:, rs],
                                             wvv[:, d, :],
                                             start=(d == 0), stop=(d == KD - 1))
                        # rotary on k [rows, 256] viewed as 4 heads x 64
                        k3 = kp[0:rows, :].rearrange("p (h e) -> p h e", e=64)
                        ctm = cos_tm[0:rows, st:st + 1, :].broadcast_to([rows, 4, 64])
                        slo = sinS_tm[0:rows, st:st + 1, 0:32].broadcast_to([rows, 4, 32])
                        shi = sinS_tm[0:rows, st:st + 1, 32:64].broadcast_to([rows, 4, 32])
                        t1 = TT([P, 2 * P])
                        t13 = t1.bitcast(F32)[0:rows].rearrange("p (h e) -> p h e", e=64)
                        nc.vector.tensor_tensor(t13, k3, ctm, ALU.mult)
                        t2 = TT([P, 2 * P])
                        t23 = t2.bitcast(F32)[0:rows].rearrange("p (h e) -> p h e", e=64)
                        nc.vector.tensor_tensor(t23[:, :, 0:32], k3[:, :, 32:64], slo, ALU.mult)
                        nc.vector.tensor_tensor(t23[:, :, 32:64], k3[:, :, 0:32], shi, ALU.mult)
                        krot = t1.bitcast(F32)[0:rows]
                        nc.vector.tensor_tensor(krot, krot, t2.bitcast(F32)[0:rows], ALU.add)
                        r1 = TT([P, 2 * P])
                        nc.scalar.activation(r1.bitcast(F32)[0:rows], krot, AF.Relu, scale=-1.0)
                        e1 = TT([P, 2 * P])
                        nc.scalar.activation(e1.bitcast(F32)[0:rows],
                                             r1.bitcast(F32)[0:rows], AF.Exp, scale=-1.0)
                        kf = TT([P, 2 * P], dt=F32R)
                        nc.vector.scalar_tensor_tensor(kf[0:rows], krot, 0.0,
                                                       e1.bitcast(F32)[0:rows],
                                                       ALU.max, ALU.add)
                        vf = TT([P, 2 * P], dt=F32R)
                        nc.scalar.copy(vf[0:rows], vp[:rows, :])
                        first = (st == 0)
                        last_st = (st == C.NST - 1)
                        for pl in range(2):
                            sl = slice(pl * P, (pl + 1) * P)
                            nc.tensor.matmul(kvx[pl][:], kf[0:rows, sl], vf[0:rows, sl],
                                             start=first, stop=last_st)
                            nc.tensor.matmul(ksp[pl][:], kf[0:rows, sl],
                                             ones_col2[0:rows, :],
                                             start=first, stop=last_st)
                    for pl in range(2):
                        mtq = 2 * rr + pl
                        nc.scalar.copy(kvbd[mtq][:], kvx[pl][:])
                        nc.vector.tensor_copy(kvbd[mtq][0:64, 64:128],
                                              zero128.bitcast(F32)[0:64, 0:64])
                        nc.vector.tensor_copy(kvbd[mtq][64:128, 0:64],
                                              zero128.bitcast(F32)[64:128, 0:64])
                        nc.vector.tensor_copy(ksel[mtq][0:64, 0:1], ksp[pl][0:64, 0:1])
                        nc.vector.tensor_copy(ksel[mtq][64:128, 1:2], ksp[pl][64:128, 0:1])
                        nc.vector.tensor_copy(ksel[mtq][0:64, 1:2],
                                              zero128.bitcast(F32)[0:64, 0:1])
                        nc.vector.tensor_copy(ksel[mtq][64:128, 0:1],
                                              zero128.bitcast(F32)[64:128, 0:1])

            # ---- attention phase: q/rotary/phi/den/Z/attn -> scr ----
            with nc.named_scope(f"l{l}_attn"):
                for mt in range(KD):
                    wqs = W([P, KD, P])
                    nc.sync.dma_start(wqs[:], wq_d[l, :, mt * P:(mt + 1) * P]
                                      .rearrange("(a p) f -> p a f", p=P).bitcast(F32R))
                    for (c0, cw) in lchunks:
                        cs = slice(c0, c0 + cw)
                        qps = PS()
                        for d in range(KD):
                            nc.tensor.matmul(qps[:, :cw], wqs[:, d, :],
                                             x[d][:, cs], start=(d == 0), stop=(d == KD - 1))
                        q_sb = TT(dt=F32R)
                        nc.scalar.copy(q_sb[:, 0:cw], qps[:, :cw])
                        rotps = PS()
                        nc.tensor.matmul(rotps[:, :cw], rsw[:], q_sb[:, 0:cw],
                                         start=True, stop=True)
                        t1 = TT()
                        nc.vector.tensor_tensor(t1.bitcast(F32)[:, 0:cw],
                                                q_sb.bitcast(F32)[:, 0:cw],
                                                cos_fm[:, cs], ALU.mult)
                        t2 = TT()
                        nc.vector.tensor_tensor(t2.bitcast(F32)[:, 0:cw],
                                                rotps[:, :cw], sinS_fm[:, cs], ALU.mult)
                        qrot = t1.bitcast(F32)[:, 0:cw]
                        nc.vector.tensor_tensor(qrot, qrot, t2.bitcast(F32)[:, 0:cw], ALU.add)
                        r1 = TT()
                        nc.scalar.activation(r1.bitcast(F32)[:, 0:cw], qrot, AF.Relu, scale=-1.0)
                        e1 = TT()
                        nc.scalar.activation(e1.bitcast(F32)[:, 0:cw], r1.bitcast(F32)[:, 0:cw],
                                             AF.Exp, scale=-1.0)
                        qf = TT(dt=F32R)
                        nc.vector.scalar_tensor_tensor(qf[:, 0:cw], qrot, 0.0,
                                                       e1.bitcast(F32)[:, 0:cw], ALU.max, ALU.add)
                        denp = PS([2, 344])
                        nc.tensor.matmul(denp[:, :cw], ksel[mt][:], qf[:, 0:cw],
                                         start=True, stop=True)
                        lnt = TT()
                        nc.scalar.activation(lnt.bitcast(F32)[0:2, 0:cw], denp[:, :cw],
                                             AF.Ln, bias=vcol("epsat")[0:2, :])
                        zrt = TT(dt=F32R)
                        nc.scalar.activation(zrt[0:2, 0:cw],
                                             lnt.bitcast(F32)[0:2, 0:cw],
                                             AF.Exp, scale=-1.0)
                        zrep = PS()
                        nc.tensor.matmul(zrep[:, :cw], e2[:], zrt[0:2, 0:cw],
                                         start=True, stop=True)
                        zrep_sb = TT()
                        nc.scalar.copy(zrep_sb.bitcast(F32)[:, 0:cw], zrep[:, :cw])
                        attnp = PS()
                        nc.tensor.matmul(attnp[:, :cw], kvbd[mt][:], qf[:, 0:cw],
                                         start=True, stop=True)
                        nc.vector.tensor_tensor(scrb[mt][:, cs], attnp[:, :cw],
                                                zrep_sb.bitcast(F32)[:, 0:cw], ALU.mult)

            # ---- Wo + residual ----
            with nc.named_scope(f"l{l}_wo"):
                for mt in range(KD):
                    wos = W([P, KD, P], dt=BF16)
                    nc.sync.dma_start(wos[:], wo_d[l, :, mt * P:(mt + 1) * P]
                                      .rearrange("(a p) f -> p a f", p=P))
                    for (c0, cw) in lchunks:
                        cs = slice(c0, c0 + cw)
                        ops = PS()
                        for d in range(KD):
                            nc.tensor.matmul(ops[:, :cw], wos[:, d, :],
                                             scrb[d][:, cs], start=(d == 0), stop=(d == KD - 1))
                        nc.vector.tensor_tensor(x[mt][:, cs], xf[mt][:, cs], ops[:, :cw], ALU.add)

            with nc.named_scope(f"l{l}_ln1"):
                ln_pass(lchunks, "ln1g", l)

            # ---- FFN ----
            with nc.named_scope(f"l{l}_ffn"):
                ffhalves = [lchunks] if last else C.half_chunks
                for hi, hcl in enumerate(ffhalves):
                    h0 = hcl[0][0]
                    for mp in range(KF):
                        w1s = W([P, KD, P])
                        nc.sync.dma_start(w1s[:], w1_d[l, :, mp * P:(mp + 1) * P]
                                          .rearrange("(a p) f -> p a f", p=P).bitcast(F32R))
                        h1t = scrb[mp // 4][:, (mp % 4) * C.h1w:(mp % 4 + 1) * C.h1w]
                        hps = [PS() for _ in hcl]
                        for d in range(KD):
                            for ci, (c0, cw) in enumerate(hcl):
                                nc.tensor.matmul(hps[ci][:, :cw], w1s[:, d, :],
                                                 x[d][:, c0:c0 + cw],
                                                 start=(d == 0), stop=(d == KD - 1))
                        for ci, (c0, cw) in enumerate(hcl):
                            nc.scalar.activation(h1t[:, c0 - h0:c0 - h0 + cw],
                                                 hps[ci][:, :cw], AF.Gelu)
                    for mt in range(KD):
                        w2a = W([P, KF // 2, P], dt=BF16)
                        nc.sync.dma_start(w2a[:], w2_d[l, 0:C.FF // 2, mt * P:(mt + 1) * P]
                                          .rearrange("(a p) f -> p a f", p=P))
                        w2b = W([P, KF // 2, P], dt=BF16)
                        nc.sync.dma_start(w2b[:], w2_d[l, C.FF // 2:C.FF, mt * P:(mt + 1) * P]
                                          .rearrange("(a p) f -> p a f", p=P))
                        fps = [PS() for _ in hcl]
                        for f in range(KF):
                            wt = w2a if f < KF // 2 else w2b
                            h1t = scrb[f // 4][:, (f % 4) * C.h1w:(f % 4 + 1) * C.h1w]
                            for ci, (c0, cw) in enumerate(hcl):
                                nc.tensor.matmul(fps[ci][:, :cw], wt[:, f % (KF // 2), :],
                                                 h1t[:, c0 - h0:c0 - h0 + cw],
                                                 start=(f == 0), stop=(f == KF - 1))
                        for ci, (c0, cw) in enumerate(hcl):
                            cs = slice(c0, c0 + cw)
                            nc.vector.tensor_tensor(x[mt][:, cs], xf[mt][:, cs],
                                                    fps[ci][:, :cw], ALU.add)

            with nc.named_scope(f"l{l}_ln2"):
                ln_pass(lchunks, "ln2g", l)
            dump(f"l{l}_ln2")

        # ---------------- final head ----------------
        with nc.named_scope("head"):
            ln_pass([(0, 2)], "gN", 0)
            for vc in range(C.V // 512):
                yps = PS([1, 512])
                for d in range(KD):
                    t = W([P, 512])
                    nc.sync.dma_start(t[:], wout_d[d * P:(d + 1) * P,
                                      vc * 512:(vc + 1) * 512].bitcast(F32R))
                    nc.tensor.matmul(yps[0:1, :], x[d][:, 0:1], t[:],
                                     start=(d == 0), stop=(d == KD - 1))
                bt = W([P, 512], dt=F32)
                nc.sync.dma_start(bt.bitcast(F32)[0:1, 0:512],
                                  bout_d[0:1, vc * 512:(vc + 1) * 512])
                yt = W([P, 512], dt=F32)
                nc.vector.tensor_tensor(yt.bitcast(F32)[0:1, 0:512], yps[0:1, :],
                                        bt.bitcast(F32)[0:1, 0:512], ALU.add)
                nc.sync.dma_start(y_d[0:1, vc * 512:(vc + 1) * 512],
                                  yt.bitcast(F32)[0:1, 0:512])

    return dd


_CACHE = {}


def _get_built(dbg=()):
    key = ("nc", tuple(dbg))
    if key not in _CACHE:
        C = Cfg()
        nc = bass.Bass("TRN2", target_bir_lowering=False, debug=False)
        build(nc, C, dbg=dbg)
        legalize_wait_counts(nc)
        _CACHE[key] = (nc, C)
    return _CACHE[key]


def kernel(_trace=False, _dbg=(), **inputs):
    nc, C = _get_built(_dbg)
    w = {k: np.asarray(v) for k, v in inputs.items()}
    hp = host_prep(C, w)
    common = {k: np.ascontiguousarray(hp[k]) for k in
              ("posbe", "cos_fm", "sinS_fm", "cos_tm", "sinS_tm", "vecs",
               "rsw", "e2", "We", "wq", "wk", "wv", "wo", "w1", "w2",
               "wout", "bout")}
    xin = np.asarray(w["inputs"], np.float32)
    B = xin.shape[0]
    in_maps = []
    for i in range(B):
        xt = np.ascontiguousarray(xin[i].T.astype(ml_dtypes.bfloat16))
        in_maps.append(dict(common, xinT=xt))
    kw = {}
    if _trace:
        kw = dict(trace=True, trace_cores=[0])
    res = bass_utils.run_bass_kernel_spmd(nc, in_maps, core_ids=list(range(B)), **kw)
    y = np.stack([res.results[i]["y"][0] for i in range(B)]).astype(np.float32)
    if _trace or _dbg:
        return y, res
    return y



# revision 12
# speedup vs baseline: 1.2672x; 1.2672x over previous
"""Self-contained Trainium2 Bass kernel for nn_Discriminator_51049981280755.

Strategy: data-parallel over batch (8 cores, 1 batch element each, no
collectives). Per core: full transformer forward; residual x kept
feature-major [D, T] resident in SBUF fp32r; linear attention via
per-head-pair block-diagonal KV matrices.

v3 vs v2 (scheduling rewrite driven by the HW trace: PE was idle 26%
and HAM-throttled to half clock for 43% of the kernel):
- kv phase: k|v weights fused into one [P,512] moving operand (halves
  LDWEIGHTS count), ksum folded into the KV accumulation via ones
  columns on vf, lag-1 software pipeline so accumulation matmuls never
  stall the PE queue.
- attn phase: two passes. Pass 1 (mt-outer) materializes phi(q) for all
  chunks into the free half of scr (bf16). Pass 2 (chunk-pipelined)
  computes all 16 head denominators in one PSUM accumulation, one
  Ln/Exp pair per chunk (was 16), then Z-replication + KV-apply
  matmuls, all bf16.
- layernorm stats (Square on DVE, column sums via PE) emitted right
  after the producing phase and chains/normalize lagged so the PE
  stream never drains; ln2 stats for the first FFN half are emitted
  between the two down-proj halves, the second half overlaps the next
  layer's kv phase.
- rotary tables bf16, per-d kv weight streaming (2KB slots), balanced
  engine assignment (Square/vf-copy on DVE, q_sb/zrep copies on ACT).
"""
import numpy as np
import ml_dtypes
import concourse.bass as bass
import concourse.tile as tile
from concourse import mybir
import concourse.bass_utils as bass_utils

bass_utils.upload_artifacts = lambda d: str(d)  # no S3 in this container

F32 = mybir.dt.float32
F32R = mybir.dt.float32r
BF16 = mybir.dt.bfloat16
AF = mybir.ActivationFunctionType
ALU = mybir.AluOpType
P = 128


def legalize_wait_counts(nc, max_waits=1):
    """This walrus build rejects instructions carrying more than one sync-wait
    command. Hoist excess waits onto preceding same-engine NoOps."""
    n_split = 0
    for fn in nc.m.functions:
        for blk in fn.blocks:
            insts = blk.instructions
            need = False
            for inst in insts:
                si = inst.sync_info
                if si is not None and len(si.on_wait) > max_waits:
                    need = True
                    break
            if not need:
                continue
            out = []
            for inst in insts:
                si = inst.sync_info
                if si is not None and len(si.on_wait) > max_waits:
                    waits = list(si.on_wait)
                    extra, keep = waits[:-max_waits], waits[-max_waits:]
                    for j in range(0, len(extra), max_waits):
                        nop = mybir.InstNoOp(
                            name=nc.get_next_instruction_name(),
                            ins=[], outs=[], text_hint="waitsplit",
                        )
                        nop.engine = inst.engine
                        nop.sync_info = mybir.SyncInfo(
                            on_wait=list(extra[j:j + max_waits]), on_update=[])
                        out.append(nop)
                        n_split += 1
                    inst.sync_info = mybir.SyncInfo(
                        on_wait=keep, on_update=list(si.on_update))
                out.append(inst)
            blk.instructions = out
    return n_split


class Cfg:
    def __init__(self, SEQ=2048, V=4096, D=1024, FF=4096, NL=8):
        self.SEQ, self.V, self.D, self.FF, self.NL = SEQ, V, D, FF, NL
        self.T = SEQ + 1
        self.KD = D // P            # d-tiles (= head pairs)
        self.KF = FF // P
        self.KV = V // P
        self.TP = 2052
        self.h1w = (2 * self.TP) // 4          # 1026 bf16 cols per h1 slice
        # balanced token chunks, all even widths (fp32r matmul ISA needs even
        # free dims); last real token 2048 plus padded columns
        self.chunks = [(0, 342), (342, 342), (684, 342),
                       (1026, 342), (1368, 340), (1708, 342)]
        assert sum(w for _, w in self.chunks) == self.T + 1
        self.half_chunks = [self.chunks[:3], self.chunks[3:]]
        self.halves = [(0, 1026), (1026, 1024)]
        nst_full = self.T // P
        self.tail_rows = self.T - nst_full * P
        self.NST = nst_full + (1 if self.tail_rows else 0)

    def vecs_layout(self):
        idx = {}
        n = 0
        for l in range(self.NL):
            for nm in ("ln1g", "ln2g"):
                for d in range(self.KD):
                    idx[(nm, l, d)] = n; n += 1
        for d in range(self.KD):
            idx[("gN", 0, d)] = n; n += 1
        for d in range(self.KD):
            idx[("cls", 0, d)] = n; n += 1
        idx[("epsln", 0, 0)] = n; n += 1
        idx[("epsat", 0, 0)] = n; n += 1
        self.NVEC = n
        return idx


def host_prep(C, weights):
    """Build host-side constant arrays. weights: dict from setup_inputs()."""
    T, TP, D = C.T, C.TP, C.D
    f32 = np.float32
    bf16 = ml_dtypes.bfloat16
    out = {}
    # rotary tables
    inv_freq = 1.0 / (10000.0 ** (np.arange(0, 64, 2, dtype=np.float64) / 64))
    freqs = np.arange(T, dtype=np.float64)[:, None] * inv_freq[None, :]   # [T, 32]
    emb = np.concatenate([freqs, freqs], axis=-1)                          # [T, 64]
    cos = np.cos(emb).astype(f32)
    sin = np.sin(emb).astype(f32)
    sinS = sin.copy()
    sinS[:, :32] *= -1.0
    # feature-major [128, TP]: row p -> dh = p % 64, col = s
    cos_fm = np.zeros((P, TP), f32)
    sinS_fm = np.zeros((P, TP), f32)
    for p in range(P):
        cos_fm[p, :T] = cos[:, p % 64]
        sinS_fm[p, :T] = sinS[:, p % 64]
    # token-major [128, NST, 64]: row p, stile j -> s = 128j + p
    cos_tm = np.zeros((P, C.NST, 64), f32)
    sinS_tm = np.zeros((P, C.NST, 64), f32)
    for j in range(C.NST):
        s0 = j * P
        rows = min(P, T - s0)
        cos_tm[:rows, j, :] = cos[s0:s0 + rows]
        sinS_tm[:rows, j, :] = sinS[s0:s0 + rows]
    out["cos_fm"] = cos_fm.astype(bf16)
    out["sinS_fm"] = sinS_fm.astype(bf16)
    out["cos_tm"] = np.ascontiguousarray(cos_tm.reshape(P, C.NST * 64)).astype(bf16)
    out["sinS_tm"] = np.ascontiguousarray(sinS_tm.reshape(P, C.NST * 64)).astype(bf16)
    # rotate-half permutation: out = Rsw.T @ q  ->  out[q_] = q[swap(q_)]
    rsw = np.zeros((P, P), f32)
    for q_ in range(P):
        blk = (q_ // 64) * 64
        r = q_ % 64
        src = blk + (r + 32 if r < 32 else r - 32)
        rsw[src, q_] = 1.0
    out["rsw"] = rsw
    # Z replication selector: e16[2*mt+j, mt, j*64:(j+1)*64] = 1
    e16 = np.zeros((16, C.KD, P), f32)
    for mt in range(C.KD):
        for j in range(2):
            e16[2 * mt + j, mt, j * 64:(j + 1) * 64] = 1.0
    out["e16"] = np.ascontiguousarray(e16.reshape(16, C.KD * P)).astype(bf16)
    # posbe [D, TP] = pos[:T].T + be
    posbe = np.zeros((D, TP), f32)
    posbe[:, :T] = np.asarray(weights["pos"][:T], f32).T + np.asarray(weights["be"], f32)[:, None]
    out["posbe"] = posbe
    # vecs
    idx = C.vecs_layout()
    vecs = np.zeros((P, C.NVEC), f32)
    ln1g = np.asarray(weights["ln1g"], f32)
    ln2g = np.asarray(weights["ln2g"], f32)
    for l in range(C.NL):
        for d in range(C.KD):
            vecs[:, idx[("ln1g", l, d)]] = ln1g[l, d * P:(d + 1) * P]
            vecs[:, idx[("ln2g", l, d)]] = ln2g[l, d * P:(d + 1) * P]
    gN = np.asarray(weights["gN"], f32)
    cls = (np.asarray(weights["We"], f32)[C.V - 1] + np.asarray(weights["be"], f32)
           + np.asarray(weights["pos"], f32)[0])
    for d in range(C.KD):
        vecs[:, idx[("gN", 0, d)]] = gN[d * P:(d + 1) * P]
        vecs[:, idx[("cls", 0, d)]] = cls[d * P:(d + 1) * P]
    vecs[:, idx[("epsln", 0, 0)]] = 1e-5
    vecs[:, idx[("epsat", 0, 0)]] = 1e-6
    out["vecs"] = vecs
    # weights
    out["We"] = np.ascontiguousarray(np.asarray(weights["We"], f32).astype(bf16))
    for nm in ("Wq", "Wk", "Wv", "W1"):
        out[nm.lower()] = np.ascontiguousarray(np.asarray(weights[nm], f32))
    out["wo"] = np.ascontiguousarray(np.asarray(weights["Wo"], f32).astype(bf16))
    out["w2"] = np.ascontiguousarray(np.asarray(weights["W2"], f32).astype(bf16))
    out["wout"] = np.ascontiguousarray(np.asarray(weights["Wout"], f32).astype(bf16))
    out["bout"] = np.ascontiguousarray(np.asarray(weights["bout"], f32)[None, :])
    return out


def build(nc, C, dbg=()):
    """Emit the full forward kernel. Returns dict of DRAM tensor handles."""
    from contextlib import ExitStack
    T, TP, D, KD, KF, KVt, NL = C.T, C.TP, C.D, C.KD, C.KF, C.KV, C.NL
    idx = C.vecs_layout()
    dd = {}

    def din(name, shape, dt=F32):
        dd[name] = nc.dram_tensor(name, shape, dt, kind="ExternalInput")
        return dd[name]

    xinT_d = din("xinT", [C.V, C.SEQ], BF16)
    We_d = din("We", [C.V, D], BF16)
    posbe_d = din("posbe", [D, TP])
    cosfm_d = din("cos_fm", [P, TP], BF16); sinSfm_d = din("sinS_fm", [P, TP], BF16)
    costm_d = din("cos_tm", [P, C.NST * 64], BF16)
    sinStm_d = din("sinS_tm", [P, C.NST * 64], BF16)
    vecs_d = din("vecs", [P, C.NVEC])
    rsw_d = din("rsw", [P, P])
    e16_d = din("e16", [16, KD * P], BF16)
    wq_d = din("wq", [NL, D, D]); wk_d = din("wk", [NL, D, D])
    wv_d = din("wv", [NL, D, D]); wo_d = din("wo", [NL, D, D], BF16)
    w1_d = din("w1", [NL, D, C.FF])
    w2_d = din("w2", [NL, C.FF, D], BF16)
    wout_d = din("wout", [D, C.V], BF16)
    bout_d = din("bout", [1, C.V])
    y_d = nc.dram_tensor("y", [1, C.V], F32, kind="ExternalOutput")
    dd["y"] = y_d
    dbg_d = {}
    for nm in dbg:
        dbg_d[nm] = nc.dram_tensor("dbg_" + nm, [C.KD * P, TP], F32,
                                   kind="ExternalOutput")
        dd["dbg_" + nm] = dbg_d[nm]

    QOFF = TP  # qf(chunk c) lives at scrb bf16 cols [QOFF+c0, QOFF+c0+cw)

    with ExitStack() as ctx:
        tc = ctx.enter_context(tile.TileContext(nc))
        big = ctx.enter_context(tc.tile_pool(name="big", bufs=1))
        wkvp = ctx.enter_context(tc.tile_pool(name="wkvp", bufs=9))
        wp = ctx.enter_context(tc.tile_pool(name="wp", bufs=5))
        tmp = ctx.enter_context(tc.tile_pool(name="tmp", bufs=8))
        pp = ctx.enter_context(tc.tile_pool(name="pp", bufs=4, space="PSUM"))

        # weight tiles: one shared rotating tag (slot = 4KB/partition)
        W = lambda shape, dt=F32R: wp.tile(shape, dt, tag="w", name="w")
        WKV = lambda: wkvp.tile([P, 512], F32R, tag="wkv", name="wkv")
        # tmp tags
        TT = lambda shape=None, dt=F32: tmp.tile(shape or [P, 344], dt, tag="t", name="t", bufs=6)
        X2 = lambda: tmp.tile([P, 344], F32R, tag="x2", name="x2", bufs=4)
        TK = lambda dt=F32: tmp.tile([P, 256], dt, tag="tk", name="tk", bufs=5)
        KFT = lambda: tmp.tile([P, 256], BF16, tag="kf", name="kf", bufs=3)
        VFT = lambda: tmp.tile([P, 2, 132], BF16, tag="vf", name="vf", bufs=3)
        ZT = lambda: tmp.tile([P, 344], BF16, tag="zt", name="zt", bufs=2)
        # PSUM tags: 4 + 2 + 2 banks
        PS = lambda shape=None: pp.tile(shape or [P, 512], F32, tag="p", name="p")
        PA = lambda shape=None: pp.tile(shape or [P, 512], F32, tag="pa", name="pa", bufs=2)
        PB = lambda shape=None: pp.tile(shape or [P, 512], F32, tag="pb", name="pb", bufs=2)

        x = [big.tile([P, TP], F32R, tag=f"x{d}", name=f"x{d}") for d in range(KD)]
        scr = [big.tile([P, TP], F32R, tag=f"scr{d}", name=f"scr{d}") for d in range(KD)]
        cos_fm = big.tile([P, TP], BF16, tag="cosfm")
        sinS_fm = big.tile([P, TP], BF16, tag="sinSfm")
        cos_tm = big.tile([P, C.NST, 64], BF16, tag="costm")
        sinS_tm = big.tile([P, C.NST, 64], BF16, tag="sinStm")
        vecs = big.tile([P, C.NVEC], F32, tag="vecs")
        rsw = big.tile([P, P], F32R, tag="rsw")
        e16 = big.tile([16, KD, P], BF16, tag="e16")
        ones_col = big.tile([P, 1], F32R, tag="ones_col")
        ones_row = big.tile([1, P], F32R, tag="ones_row")
        kvbd = [big.tile([P, P], BF16, tag=f"kvbd{p_}", name=f"kvbd{p_}") for p_ in range(KD)]
        ksel16 = [big.tile([P, 16], BF16, tag=f"ksel{p_}", name=f"ksel{p_}") for p_ in range(KD)]
        xcls_bf = big.tile([P, KD], BF16, tag="xclsbf")

        nc.sync.dma_start(cos_fm[:], cosfm_d[:])
        nc.sync.dma_start(sinS_fm[:], sinSfm_d[:])
        nc.sync.dma_start(cos_tm[:], costm_d.rearrange("p (j e) -> p j e", e=64))
        nc.sync.dma_start(sinS_tm[:], sinStm_d.rearrange("p (j e) -> p j e", e=64))
        nc.sync.dma_start(vecs[:], vecs_d[:])
        nc.sync.dma_start(rsw[:], rsw_d[:].bitcast(F32R))
        nc.sync.dma_start(e16[:], e16_d.rearrange("p (a f) -> p a f", f=P))
        nc.vector.memset(ones_col.bitcast(F32)[:], 1.0)
        nc.vector.memset(ones_row.bitcast(F32)[:], 1.0)
        for d in range(KD):
            nc.vector.memset(x[d].bitcast(F32)[:, T:TP], 0.0)

        vcol = lambda nm, l=0, d=0: vecs[:, idx[(nm, l, d)]:idx[(nm, l, d)] + 1]

        def dump(nm):
            if nm not in dbg_d:
                return
            t = dbg_d[nm]
            for d in range(KD):
                nc.sync.dma_start(t[d * P:(d + 1) * P, :], x[d].bitcast(F32)[:])

        xf = [t.bitcast(F32) for t in x]
        scrb = [t.bitcast(BF16) for t in scr]

        def ln_pass(chunks_list, gname, l):
            """in-place layernorm on x over the given column chunks.
            Stats matmuls stream first (lag-free for the PE); chains and
            normalization trail chunk-by-chunk."""
            invD = 1.0 / D

            def emit_stats(c0, cw):
                cs = slice(c0, c0 + cw)
                sums = PB(); sqs = PB()
                for d in range(KD):
                    nc.tensor.matmul(sums[0:1, 0:cw], ones_col[:], x[d][:, cs],
                                     start=(d == 0), stop=(d == KD - 1))
                # Square on DVE (scalar engine is loaded elsewhere)
                for d in range(KD):
                    x2t = X2()
                    nc.vector.tensor_tensor(x2t[:, 0:cw], xf[d][:, cs],
                                            xf[d][:, cs], ALU.mult)
                    nc.tensor.matmul(sqs[0:1, 0:cw], ones_col[:],
                                     x2t[:, 0:cw], start=(d == 0), stop=(d == KD - 1))
                return c0, cw, sums, sqs

            def emit_chain(c0, cw, sums, sqs):
                meant = TT(); m2tt = TT(); vart = TT()
                mean = meant.bitcast(F32)[0:1, 0:cw]
                m2t = m2tt.bitcast(F32)[0:1, 0:cw]
                var = vart.bitcast(F32)[0:1, 0:cw]
                nc.vector.tensor_scalar_mul(mean, sums[0:1, 0:cw], invD)
                nc.scalar.activation(m2t, mean, AF.Square)
                nc.vector.scalar_tensor_tensor(var, sqs[0:1, 0:cw], invD,
                                               m2t, ALU.mult, ALU.subtract)
                rstdt = TT(dt=F32R); m2rt = TT(dt=F32R); lnvt = TT()
                rstdf = rstdt.bitcast(F32)[0:1, 0:cw]
                rstd = rstdt[0:1, 0:cw]
                m2 = m2rt[0:1, 0:cw]
                lnv = lnvt.bitcast(F32)[0:1, 0:cw]
                nc.scalar.activation(lnv, var, AF.Ln, bias=vcol("epsln")[0:1, :])
                nc.scalar.activation(rstd, lnv, AF.Exp, scale=-0.5)
                nc.vector.scalar_tensor_tensor(m2, mean, -1.0, rstdf,
                                               ALU.mult, ALU.mult)
                Rrep = PA(); Mrep = PS()
                nc.tensor.matmul(Rrep[:, 0:cw], ones_row[:], rstd, start=True, stop=True)
                nc.tensor.matmul(Mrep[:, 0:cw], ones_row[:], m2, start=True, stop=True)
                return c0, cw, Rrep, Mrep

            def emit_norm(c0, cw, Rrep, Mrep):
                cs = slice(c0, c0 + cw)
                for d in range(KD):
                    g = vcol(gname, l, d)
                    t1 = TT()
                    nc.vector.scalar_tensor_tensor(t1.bitcast(F32)[:, 0:cw], xf[d][:, cs], g,
                                                   Rrep[:, 0:cw], ALU.mult, ALU.mult)
                    nc.vector.scalar_tensor_tensor(x[d][:, cs], Mrep[:, 0:cw], g,
                                                   t1.bitcast(F32)[:, 0:cw], ALU.mult, ALU.add)

            p1, p2 = [], []
            for (c0, cw) in chunks_list:
                p1.append(emit_stats(c0, cw))
                if len(p1) > 1:
                    p2.append(emit_chain(*p1.pop(0)))
                if len(p2) > 1:
                    emit_norm(*p2.pop(0))
            while p1:
                p2.append(emit_chain(*p1.pop(0)))
                if len(p2) > 1:
                    emit_norm(*p2.pop(0))
            while p2:
                emit_norm(*p2.pop(0))

        # ---------------- embedding ----------------
        with nc.named_scope("emb"):
            # We resident as bf16 in the (currently unused) scr region:
            # We tile vt -> scr[vt//4] bf16 cols [(vt%4)*1024, +1024)
            for vt in range(KVt):
                nc.sync.dma_start(
                    scrb[vt // 4][:, (vt % 4) * 1024:(vt % 4 + 1) * 1024],
                    We_d[vt * P:(vt + 1) * P, :])
            for d in range(KD):
                nc.scalar.copy(x[d][:, 0:1], vcol("cls", 0, d))
            for sc in range(C.SEQ // 512):
                cs = slice(1 + sc * 512, 1 + (sc + 1) * 512)
                for g in range(2):
                    embps = [PS() for _ in range(4)]
                    for vt in range(KVt):
                        xt = wp.tile([P, 512], BF16, tag="w", name="xt")
                        nc.sync.dma_start(xt[:], xinT_d[vt * P:(vt + 1) * P,
                                          sc * 512:(sc + 1) * 512])
                        for dg in range(4):
                            d = g * 4 + dg
                            wsl = scrb[vt // 4][:, (vt % 4) * 1024 + d * P:
                                               (vt % 4) * 1024 + (d + 1) * P]
                            nc.tensor.matmul(embps[dg][:, :], wsl, xt[:],
                                             start=(vt == 0), stop=(vt == KVt - 1))
                    for dg in range(4):
                        d = g * 4 + dg
                        pb = wp.tile([P, 512], F32, tag="w", name="pb")
                        nc.sync.dma_start(pb[:], posbe_d[d * P:(d + 1) * P, cs])
                        nc.vector.tensor_tensor(x[d][:, cs], embps[dg][:, :],
                                                pb[:], ALU.add)

        dump("emb")
        # ---------------- layers ----------------
        for l in range(NL):
            last = (l == NL - 1)
            lchunks = [(0, 2)] if last else C.chunks

            # ---- KV phase: k/v token-major -> kvbd (blockdiag) + ksel16 ----
            with nc.named_scope(f"l{l}_kv"):
                for mt in range(KD):
                    nc.vector.memset(ksel16[mt][:], 0.0)
                for rr in range(KD // 2):
                    # fused k|v weights: wkv[d][:, 0:256] = wk cols of the two
                    # mt tiles, [:, 256:512] = wv cols
                    wkv = []
                    for d in range(KD):
                        t = WKV()
                        for half in range(2):
                            mtq = 2 * rr + half
                            nc.sync.dma_start(
                                t[:, half * P:(half + 1) * P],
                                wk_d[l, d * P:(d + 1) * P, mtq * P:(mtq + 1) * P]
                                .bitcast(F32R))
                            nc.sync.dma_start(
                                t[:, 256 + half * P:256 + (half + 1) * P],
                                wv_d[l, d * P:(d + 1) * P, mtq * P:(mtq + 1) * P]
                                .bitcast(F32R))
                        wkv.append(t)
                    kvx = [PA([P, 132]) for _ in range(2)]
                    pend = []

                    def emit_kpvp(st):
                        rows = P if st * P + P <= T else T - st * P
                        rs = slice(st * P, st * P + rows)
                        kpvp = PS()
                        for d in range(KD):
                            nc.tensor.matmul(kpvp[:rows, :], x[d][:, rs],
                                             wkv[d][:], start=(d == 0),
                                             stop=(d == KD - 1))
                        return st, rows, kpvp

                    def emit_chain(st, rows, kpvp):
                        # rotary on k [rows, 256] viewed as 4 heads x 64
                        k3 = kpvp[0:rows, 0:256].rearrange("p (h e) -> p h e", e=64)
                        ctm = cos_tm[0:rows, st:st + 1, :].broadcast_to([rows, 4, 64])
                        slo = sinS_tm[0:rows, st:st + 1, 0:32].broadcast_to([rows, 4, 32])
                        shi = sinS_tm[0:rows, st:st + 1, 32:64].broadcast_to([rows, 4, 32])
                        t1 = TK()
                        t13 = t1.bitcast(F32)[0:rows].rearrange("p (h e) -> p h e", e=64)
                        nc.vector.tensor_tensor(t13, k3, ctm, ALU.mult)
                        t2 = TK()
                        t23 = t2.bitcast(F32)[0:rows].rearrange("p (h e) -> p h e", e=64)
                        nc.vector.tensor_tensor(t23[:, :, 0:32], k3[:, :, 32:64], slo, ALU.mult)
                        nc.vector.tensor_tensor(t23[:, :, 32:64], k3[:, :, 0:32], shi, ALU.mult)
                        krot = t1.bitcast(F32)[0:rows]
                        nc.vector.tensor_tensor(krot, krot, t2.bitcast(F32)[0:rows], ALU.add)
                        r1 = TK()
                        nc.scalar.activation(r1.bitcast(F32)[0:rows], krot, AF.Relu, scale=-1.0)
                        e1 = TK()
                        nc.scalar.activation(e1.bitcast(F32)[0:rows],
                                             r1.bitcast(F32)[0:rows], AF.Exp, scale=-1.0)
                        kf = KFT()
                        nc.vector.scalar_tensor_tensor(kf[0:rows], krot, 0.0,
                                                       e1.bitcast(F32)[0:rows],
                                                       ALU.max, ALU.add)
                        vf = VFT()
                        for pl in range(2):
                            nc.vector.tensor_copy(vf[0:rows, pl, 0:128],
                                                  kpvp[0:rows, 256 + pl * 128:256 + (pl + 1) * 128])
                            nc.vector.memset(vf[0:rows, pl, 128:130], 1.0)
                        first = (st == 0)
                        last_st = (st == C.NST - 1)
                        for pl in range(2):
                            nc.tensor.matmul(kvx[pl][:, 0:130],
                                             kf[0:rows, pl * P:(pl + 1) * P],
                                             vf[0:rows, pl, 0:130],
                                             start=first, stop=last_st)

                    for st in range(C.NST):
                        pend.append(emit_kpvp(st))
                        if len(pend) > 1:
                            emit_chain(*pend.pop(0))
                    while pend:
                        emit_chain(*pend.pop(0))

                    for pl in range(2):
                        mtq = 2 * rr + pl
                        nc.scalar.copy(kvbd[mtq][:], kvx[pl][:, 0:128])
                        nc.vector.memset(kvbd[mtq][0:64, 64:128], 0.0)
                        nc.vector.memset(kvbd[mtq][64:128, 0:64], 0.0)
                        nc.scalar.copy(ksel16[mtq][0:64, 2 * mtq:2 * mtq + 1],
                                       kvx[pl][0:64, 128:129])
                        nc.scalar.copy(ksel16[mtq][64:128, 2 * mtq + 1:2 * mtq + 2],
                                       kvx[pl][64:128, 128:129])

            # ---- attn pass 1: phi(rot(q)) for all mt/chunks -> scrb hi ----
            with nc.named_scope(f"l{l}_attn"):
                for mt in range(KD):
                    wqs = W([P, KD, P])
                    nc.sync.dma_start(wqs[:], wq_d[l, :, mt * P:(mt + 1) * P]
                                      .rearrange("(a p) f -> p a f", p=P).bitcast(F32R))
                    pend = []

                    def emit_qps(c0, cw):
                        qps = PS()
                        for d in range(KD):
                            nc.tensor.matmul(qps[:, :cw], wqs[:, d, :],
                                             x[d][:, c0:c0 + cw],
                                             start=(d == 0), stop=(d == KD - 1))
                        q_sb = TT(dt=F32R)
                        nc.scalar.copy(q_sb[:, 0:cw], qps[:, :cw])
                        rotps = PS()
                        nc.tensor.matmul(rotps[:, :cw], rsw[:], q_sb[:, 0:cw],
                                         start=True, stop=True)
                        return c0, cw, q_sb, rotps

                    def emit_qchain(c0, cw, q_sb, rotps):
                        cs = slice(c0, c0 + cw)
                        t1 = TT()
                        nc.vector.tensor_tensor(t1.bitcast(F32)[:, 0:cw],
                                                q_sb.bitcast(F32)[:, 0:cw],
                                                cos_fm[:, cs], ALU.mult)
                        t2 = TT()
                        nc.vector.tensor_tensor(t2.bitcast(F32)[:, 0:cw],
                                                rotps[:, :cw], sinS_fm[:, cs], ALU.mult)
                        qrot = t1.bitcast(F32)[:, 0:cw]
                        nc.vector.tensor_tensor(qrot, qrot, t2.bitcast(F32)[:, 0:cw], ALU.add)
                        r1 = TT()
                        nc.scalar.activation(r1.bitcast(F32)[:, 0:cw], qrot, AF.Relu, scale=-1.0)
                        e1 = TT()
                        nc.scalar.activation(e1.bitcast(F32)[:, 0:cw], r1.bitcast(F32)[:, 0:cw],
                                             AF.Exp, scale=-1.0)
                        nc.vector.scalar_tensor_tensor(
                            scrb[mt][:, QOFF + c0:QOFF + c0 + cw], qrot, 0.0,
                            e1.bitcast(F32)[:, 0:cw], ALU.max, ALU.add)

                    for (c0, cw) in lchunks:
                        pend.append(emit_qps(c0, cw))
                        if len(pend) > 1:
                            emit_qchain(*pend.pop(0))
                    while pend:
                        emit_qchain(*pend.pop(0))

                # ---- attn pass 2: den16 -> Z -> apply KV, chunk-pipelined ----
                pend = []

                def emit_den(c0, cw):
                    den = PA([16, 344])
                    for mt in range(KD):
                        nc.tensor.matmul(den[:, 0:cw], ksel16[mt][:],
                                         scrb[mt][:, QOFF + c0:QOFF + c0 + cw],
                                         start=(mt == 0), stop=(mt == KD - 1))
                    lnt = TT()
                    nc.scalar.activation(lnt.bitcast(F32)[0:16, 0:cw], den[:, 0:cw],
                                         AF.Ln, bias=vcol("epsat")[0:16, :])
                    zrt = ZT()
                    nc.scalar.activation(zrt[0:16, 0:cw],
                                         lnt.bitcast(F32)[0:16, 0:cw], AF.Exp, scale=-1.0)
                    return c0, cw, zrt

                def emit_apply(c0, cw, zrt):
                    cs = slice(c0, c0 + cw)
                    for mt in range(KD):
                        zrep = PS()
                        nc.tensor.matmul(zrep[:, :cw], e16[:, mt, :], zrt[0:16, 0:cw],
                                         start=True, stop=True)
                        attnp = PS()
                        nc.tensor.matmul(attnp[:, :cw], kvbd[mt][:],
                                         scrb[mt][:, QOFF + c0:QOFF + c0 + cw],
                                         start=True, stop=True)
                        zrep_sb = TT()
                        nc.scalar.copy(zrep_sb.bitcast(F32)[:, 0:cw], zrep[:, :cw])
                        nc.vector.tensor_tensor(scrb[mt][:, cs], attnp[:, :cw],
                                                zrep_sb.bitcast(F32)[:, 0:cw], ALU.mult)

                for (c0, cw) in lchunks:
                    pend.append(emit_den(c0, cw))
                    if len(pend) > 1:
                        emit_apply(*pend.pop(0))
                while pend:
                    emit_apply(*pend.pop(0))

            # ---- Wo + residual ----
            with nc.named_scope(f"l{l}_wo"):
                for mt in range(KD):
                    wos = W([P, KD, P], dt=BF16)
                    nc.sync.dma_start(wos[:], wo_d[l, :, mt * P:(mt + 1) * P]
                                      .rearrange("(a p) f -> p a f", p=P))
                    for (c0, cw) in lchunks:
                        cs = slice(c0, c0 + cw)
                        ops = PS()
                        for d in range(KD):
                            nc.tensor.matmul(ops[:, :cw], wos[:, d, :],
                                             scrb[d][:, cs], start=(d == 0), stop=(d == KD - 1))
                        nc.vector.tensor_tensor(x[mt][:, cs], xf[mt][:, cs], ops[:, :cw], ALU.add)

            with nc.named_scope(f"l{l}_ln1"):
                ln_pass(lchunks, "ln1g", l)

            # ---- FFN ----
            with nc.named_scope(f"l{l}_ffn"):
                ffhalves = [lchunks] if last else C.half_chunks
                for hi, hcl in enumerate(ffhalves):
                    h0 = hcl[0][0]
                    for mp in range(KF):
                        w1s = W([P, KD, P])
                        nc.sync.dma_start(w1s[:], w1_d[l, :, mp * P:(mp + 1) * P]
                                          .rearrange("(a p) f -> p a f", p=P).bitcast(F32R))
                        h1t = scrb[mp // 4][:, (mp % 4) * C.h1w:(mp % 4 + 1) * C.h1w]
                        hps = [PS() for _ in hcl]
                        for d in range(KD):
                            for ci, (c0, cw) in enumerate(hcl):
                                nc.tensor.matmul(hps[ci][:, :cw], w1s[:, d, :],
                                                 x[d][:, c0:c0 + cw],
                                                 start=(d == 0), stop=(d == KD - 1))
                        for ci, (c0, cw) in enumerate(hcl):
                            nc.scalar.activation(h1t[:, c0 - h0:c0 - h0 + cw],
                                                 hps[ci][:, :cw], AF.Gelu)
                    for mt in range(KD):
                        w2a = W([P, KF // 2, P], dt=BF16)
                        nc.sync.dma_start(w2a[:], w2_d[l, 0:C.FF // 2, mt * P:(mt + 1) * P]
                                          .rearrange("(a p) f -> p a f", p=P))
                        w2b = W([P, KF // 2, P], dt=BF16)
                        nc.sync.dma_start(w2b[:], w2_d[l, C.FF // 2:C.FF, mt * P:(mt + 1) * P]
                                          .rearrange("(a p) f -> p a f", p=P))
                        fps = [PS() for _ in hcl]
                        for f in range(KF):
                            wt = w2a if f < KF // 2 else w2b
                            h1t = scrb[f // 4][:, (f % 4) * C.h1w:(f % 4 + 1) * C.h1w]
                            for ci, (c0, cw) in enumerate(hcl):
                                nc.tensor.matmul(fps[ci][:, :cw], wt[:, f % (KF // 2), :],
                                                 h1t[:, c0 - h0:c0 - h0 + cw],
                                                 start=(f == 0), stop=(f == KF - 1))
                        for ci, (c0, cw) in enumerate(hcl):
                            cs = slice(c0, c0 + cw)
                            nc.vector.tensor_tensor(x[mt][:, cs], xf[mt][:, cs],
                                                    fps[ci][:, :cw], ALU.add)
                    # ln2 for this half's chunks: first half's stats overlap
                    # the second half's down-proj, second half's overlap the
                    # next layer's kv phase
                    with nc.named_scope(f"l{l}_ln2"):
                        if last:
                            ln_pass(lchunks, "ln2g", l)
                        else:
                            ln_pass(C.half_chunks[hi], "ln2g", l)
            dump(f"l{l}_ln2")

        # ---------------- final head ----------------
        with nc.named_scope("head"):
            ln_pass([(0, 2)], "gN", 0)
            for d in range(KD):
                nc.vector.tensor_copy(xcls_bf[:, d:d + 1], xf[d][:, 0:1])
            for vc in range(C.V // 512):
                yps = PS([1, 512])
                for d in range(KD):
                    t = wp.tile([P, 512], BF16, tag="w", name="wt")
                    nc.sync.dma_start(t[:], wout_d[d * P:(d + 1) * P,
                                      vc * 512:(vc + 1) * 512])
                    nc.tensor.matmul(yps[0:1, :], xcls_bf[:, d:d + 1], t[:],
                                     start=(d == 0), stop=(d == KD - 1))
                bt = wp.tile([P, 512], F32, tag="w", name="bt")
                nc.sync.dma_start(bt.bitcast(F32)[0:1, 0:512],
                                  bout_d[0:1, vc * 512:(vc + 1) * 512])
                yt = wp.tile([P, 512], F32, tag="w", name="yt")
                nc.vector.tensor_tensor(yt.bitcast(F32)[0:1, 0:512], yps[0:1, :],
                                        bt.bitcast(F32)[0:1, 0:512], ALU.add)
                nc.sync.dma_start(y_d[0:1, vc * 512:(vc + 1) * 512],
                                  yt.bitcast(F32)[0:1, 0:512])

    return dd


_CACHE = {}


def _get_built(dbg=()):
    key = ("nc", tuple(dbg))
    if key not in _CACHE:
        C = Cfg()
        nc = bass.Bass("TRN2", target_bir_lowering=False, debug=False)
        build(nc, C, dbg=dbg)
        legalize_wait_counts(nc)
        _CACHE[key] = (nc, C)
    return _CACHE[key]


def kernel(_trace=False, _dbg=(), **inputs):
    nc, C = _get_built(_dbg)
    w = {k: np.asarray(v) for k, v in inputs.items()}
    hp = host_prep(C, w)
    common = {k: np.ascontiguousarray(hp[k]) for k in
              ("posbe", "cos_fm", "sinS_fm", "cos_tm", "sinS_tm", "vecs",
               "rsw", "e16", "We", "wq", "wk", "wv", "wo", "w1", "w2",
               "wout", "bout")}
    xin = np.asarray(w["inputs"], np.float32)
    B = xin.shape[0]
    in_maps = []
    for i in range(B):
        xt = np.ascontiguousarray(xin[i].T.astype(ml_dtypes.bfloat16))
        in_maps.append(dict(common, xinT=xt))
    kw = {}
    if _trace:
        kw = dict(trace=True, trace_cores=[0])
    res = bass_utils.run_bass_kernel_spmd(nc, in_maps, core_ids=list(range(B)), **kw)
    y = np.stack([res.results[i]["y"][0] for i in range(B)]).astype(np.float32)
    if _trace or _dbg:
        return y, res
    return y
